# revision 10
# baseline (speedup 1.0000x reference)
# PhiAttention Trainium2 kernel: 8-core DP(batch=2) x TP(4 head-groups of 8 heads).
# kernel() takes full inputs, shards on host, runs one Bass program SPMD on 8
# NeuronCores, and gathers/reduces the partial outputs on host.
#
# Per-core layout notes:
#  - q/k head dims are host-permuted to [rot1(16) | pass(16) | rot2(16) |
#    pass(32) | zero-pad(16)] = 96 rows per head, so every partition slice the
#    engines touch starts at a 32-aligned quadrant boundary.
#  - scores are computed transposed (s^T[k,q]) so softmax denominators come from
#    a ones-column appended to v, and P@V needs no transposes at all.
import numpy as np
from contextlib import ExitStack

import concourse.bass as bass
import concourse.tile as tile
import concourse.mybir as mybir
from concourse import bacc
from concourse.bass_utils import run_bass_kernel_spmd

F32 = mybir.dt.float32
F32R = mybir.dt.float32r
ALU = mybir.AluOpType
ACT_EXP = mybir.ActivationFunctionType.Exp

HIDDEN = 2560
S = 2048
NH = 8            # heads per core
HP = 96           # padded head rows for q/k and attn
KC = 20           # 2560/128 contraction chunks
NCH = 4           # s-chunks
CH = 512
NQK = 12          # q/k douttiles (2*8*96/128)
SCALE = 1.0 / float(np.sqrt(80.0))


def _build():
    nc = bacc.Bacc("TRN2", target_bir_lowering=False, debug=False, num_devices=8)

    xT = nc.dram_tensor("xT", [HIDDEN, S], F32R, kind="ExternalInput")
    wqk = nc.dram_tensor("wqk", [HIDDEN, 1536], F32R, kind="ExternalInput")
    wv = nc.dram_tensor("wv", [HIDDEN, 640], F32R, kind="ExternalInput")
    wd = nc.dram_tensor("wd", [768, HIDDEN], F32R, kind="ExternalInput")
    bqk = nc.dram_tensor("bqk", [128, NQK], F32, kind="ExternalInput")
    bv = nc.dram_tensor("bv", [1, 640], F32, kind="ExternalInput")
    cosT = nc.dram_tensor("cosT", [16, S], F32, kind="ExternalInput")
    sinT = nc.dram_tensor("sinT", [16, S], F32, kind="ExternalInput")
    maskb = nc.dram_tensor("maskb", [128, 896], F32, kind="ExternalInput")
    vpad = nc.dram_tensor("vpad", [128, NH * 17], F32R, kind="ExternalInput")
    outp = nc.dram_tensor("outp", [S, HIDDEN], F32, kind="ExternalOutput")

    xT_r = xT[:].rearrange("(ko ki) s -> ki ko s", ki=128)      # [128,20,2048]
    wqk_r = wqk[:].rearrange("(ko ki) n -> ki ko n", ki=128)    # [128,20,1536]
    wv_r = wv[:].rearrange("(ko ki) n -> ki ko n", ki=128)      # [128,20,640]
    wd_r = wd[:].rearrange("(ko ki) n -> ki ko n", ki=128)      # [128,6,2560]

    with tile.TileContext(nc) as tc, ExitStack() as ctx:
        persist = ctx.enter_context(tc.tile_pool(name="persist", bufs=1))
        xt_pool = ctx.enter_context(tc.tile_pool(name="xt", bufs=1))
        w_pool = ctx.enter_context(tc.tile_pool(name="w", bufs=2))
        wd_pool = ctx.enter_context(tc.tile_pool(name="wdp", bufs=1))
        q_pool = ctx.enter_context(tc.tile_pool(name="q", bufs=1))
        work = ctx.enter_context(tc.tile_pool(name="work", bufs=2))
        workA = ctx.enter_context(tc.tile_pool(name="workA", bufs=1))
        ex_pool = ctx.enter_context(tc.tile_pool(name="ex", bufs=2))
        at_pool = ctx.enter_context(tc.tile_pool(name="at", bufs=1))
        kread = ctx.enter_context(tc.tile_pool(name="kread", bufs=3))
        dram = ctx.enter_context(tc.tile_pool(name="dram", bufs=1, space="DRAM"))
        ps_proj = ctx.enter_context(tc.tile_pool(name="psp", bufs=2, space="PSUM"))
        ps_sc = ctx.enter_context(tc.tile_pool(name="pssc", bufs=2, space="PSUM"))
        ps_pv = ctx.enter_context(tc.tile_pool(name="pspv", bufs=2, space="PSUM"))
        ps_wd = ctx.enter_context(tc.tile_pool(name="pswd", bufs=2, space="PSUM"))

        # persistent state
        ktd = dram.tile([NH * HP, S], F32R, name="ktd")
        vt = [persist.tile([128, NH, 97], F32R, tag=f"vt{i}", name=f"vt{i}")
              for i in range(16)]
        mask_sb = persist.tile([128, 896], F32, tag="mask")
        nc.sync.dma_start(mask_sb[:], maskb[:])
        bqk_sb = persist.tile([128, NQK], F32, tag="bqk")
        nc.sync.dma_start(bqk_sb[:], bqk[:])
        bvb = persist.tile([128, 640], F32, tag="bvb")
        bv1 = work.tile([1, 640], F32, tag="zb")
        nc.sync.dma_start(bv1[:], bv[:])
        nc.gpsimd.partition_broadcast(bvb[:], bv1[:])
        vpad_r = vpad[:].rearrange("p (h z) -> p h z", z=17)
        for i in range(16):
            nc.sync.dma_start(vt[i][:, :, 80:97], vpad_r)

        for c in range(NCH):
            cs = bass.ds(c * CH, CH)
            xt = xt_pool.tile([128, KC, CH], F32R, tag="xt")
            nc.sync.dma_start(xt[:], xT_r[:, :, cs])

            # cos/sin replicated at every quadrant base (tensor_tensor needs
            # equal input base partitions when both operands are in SBUF)
            cos_c = workA.tile([112, CH], F32, tag="cos")
            sin_c = workA.tile([112, CH], F32, tag="sin")
            for b in (0, 32, 64, 96):
                nc.sync.dma_start(cos_c[b:b + 16, :], cosT[:, cs])
                nc.sync.dma_start(sin_c[b:b + 16, :], sinT[:, cs])

            # ---- q/k projection (transposed orientation), 12 douttiles
            qh = [q_pool.tile([HP, CH], F32R, tag=f"qh{t}", name=f"qh{t}")
                  for t in range(NH)]
            kst = [q_pool.tile([128, CH], F32R, tag=f"kst{t}", name=f"kst{t}")
                   for t in range(6)]
            for t in range(NQK):
                ps = ps_proj.tile([128, CH], F32, tag="proj")
                for kg in range(4):
                    wtile = w_pool.tile([128, 5, 128], F32R, tag="wqk")
                    nc.sync.dma_start(
                        wtile[:], wqk_r[:, bass.ds(kg * 5, 5), bass.ts(t, 128)])
                    for kk in range(5):
                        k = kg * 5 + kk
                        nc.tensor.matmul(ps[:], wtile[:, kk], xt[:, k],
                                         start=(k == 0), stop=(k == KC - 1))
                if t < 6:
                    # scatter 32-row granules into per-head padded q tiles
                    for g in range(4):
                        r = 128 * t + 32 * g
                        h = r // HP
                        off = r - HP * h
                        nc.vector.tensor_scalar(qh[h][off:off + 32, :],
                                                ps[32 * g:32 * g + 32, :],
                                                bqk_sb[32 * g:32 * g + 32, t:t + 1],
                                                SCALE, ALU.add, ALU.mult)
                else:
                    nc.vector.tensor_scalar(kst[t - 6][:], ps[:],
                                            bqk_sb[:, t:t + 1], 1.0,
                                            ALU.add, ALU.mult)

            # ---- v projection (natural orientation), two m-passes
            for nh2 in range(2):
                for mp in range(2):
                    pss = [ps_proj.tile([128, CH], F32, tag="proj", name=f"vps{i}")
                           for i in range(2)]
                    for kg in range(4):
                        wvt = w_pool.tile([128, 5, 320], F32R, tag="wv")
                        nc.sync.dma_start(
                            wvt[:], wv_r[:, bass.ds(kg * 5, 5),
                                         bass.ds(nh2 * 320, 320)])
                        for kk in range(5):
                            k = kg * 5 + kk
                            for mi in range(2):
                                m = mp * 2 + mi
                                nc.tensor.matmul(pss[mi][:, :320],
                                                 xt[:, k, bass.ts(m, 128)],
                                                 wvt[:, kk],
                                                 start=(k == 0), stop=(k == KC - 1))
                    for mi in range(2):
                        sti = c * 4 + mp * 2 + mi
                        for hh in range(4):
                            h = nh2 * 4 + hh
                            nc.vector.tensor_tensor(vt[sti][:, h, 0:80],
                                                    pss[mi][:, bass.ds(hh * 80, 80)],
                                                    bvb[:, bass.ds(h * 80, 80)],
                                                    ALU.add)

            # ---- RoPE: rot1 rows [0:16], rot2 rows [32:48] of each 96-row head
            def rope(get, pb):
                # get(base, ln) -> AP at head rows [base, base+ln); pb(base) ->
                # physical base partition of that slice (for cos/sin alignment)
                tmp = workA.tile([16, 4, CH], F32, tag="ropetmp")
                b0, b2 = pb(0), pb(32)
                nc.vector.tensor_tensor(tmp[:, 0, :], get(32, 16),
                                        sin_c[b2:b2 + 16, :], ALU.mult)
                nc.vector.tensor_tensor(tmp[:, 1, :], get(0, 16),
                                        sin_c[b0:b0 + 16, :], ALU.mult)
                nc.vector.tensor_tensor(tmp[:, 2, :], get(0, 16),
                                        cos_c[b0:b0 + 16, :], ALU.mult)
                nc.vector.tensor_tensor(tmp[:, 3, :], get(32, 16),
                                        cos_c[b2:b2 + 16, :], ALU.mult)
                nc.vector.tensor_tensor(get(0, 16), tmp[:, 2, :], tmp[:, 0, :],
                                        ALU.subtract)
                nc.vector.tensor_tensor(get(32, 16), tmp[:, 3, :], tmp[:, 1, :],
                                        ALU.add)

            for h in range(NH):
                rope(lambda base, ln, h=h: qh[h][base:base + ln, :],
                     lambda base: base)
            for h in range(NH):
                r0 = HP * h
                rope(lambda base, ln, r0=r0: kst[(r0 + base) // 128][
                    (r0 + base) % 128:(r0 + base) % 128 + ln, :],
                     lambda base, r0=r0: (r0 + base) % 128)
            for t in range(6):
                nc.sync.dma_start(ktd[bass.ts(t, 128), cs], kst[t][:])

            # ---- causal attention for this q-chunk (scores transposed: [k,q])
            att = [at_pool.tile([128, CH], F32R, tag=f"at{t}", name=f"at{t}")
                   for t in range(6)]
            nkc = 4 * c + 4
            for h in range(NH):
                pv = ps_pv.tile([128, CH], F32, tag="pv")
                for cc in range(c + 1):
                    kld = kread.tile([HP, CH], F32R, tag="kld")
                    nc.sync.dma_start(kld[:], ktd[bass.ds(HP * h, HP),
                                                  bass.ds(cc * CH, CH)])
                    for kq in range(4):
                        kc = cc * 4 + kq
                        sp = ps_sc.tile([128, CH], F32, tag="sc")
                        nc.tensor.matmul(sp[:], kld[:, bass.ts(kq, 128)], qh[h][:],
                                         start=True, stop=True)
                        ex = ex_pool.tile([128, CH], F32R, tag="ex")
                        nc.scalar.activation(ex[:], sp[:], ACT_EXP)
                        m = kc - 4 * c
                        if m >= 0:
                            nc.vector.tensor_tensor(
                                ex[:], ex[:],
                                mask_sb[:, bass.ds(384 - 128 * m, CH)], ALU.mult)
                        nc.tensor.matmul(pv[0:97, :], vt[kc][:, h, :], ex[:],
                                         start=(kc == 0), stop=(kc == nkc - 1))
                # normalize rows 0..95 by reciprocal of row 96 (softmax denom)
                zr = work.tile([1, CH], F32, tag="zr")
                nc.vector.reciprocal(zr[:], pv[96:97, :])
                zb = work.tile([128, CH], F32, tag="zb")
                nc.gpsimd.partition_broadcast(zb[0:96, :], zr[:])
                for a in range(3):            # 32-row granules of the 96 rows
                    r = HP * h + 32 * a
                    t, boff = r // 128, r % 128
                    nc.vector.tensor_tensor(att[t][boff:boff + 32, :],
                                            pv[32 * a:32 * a + 32, :],
                                            zb[32 * a:32 * a + 32, :], ALU.mult)

            # ---- output projection (row-parallel slice of Wd, zero-padded rows)
            for n in range(5):
                wdt = wd_pool.tile([128, 6, CH], F32R, tag="wd")
                nc.sync.dma_start(wdt[:], wd_r[:, :, bass.ts(n, CH)])
                for m in range(4):
                    ps = ps_wd.tile([128, CH], F32, tag="wdp")
                    for t in range(6):
                        nc.tensor.matmul(ps[:], att[t][:, bass.ts(m, 128)],
                                         wdt[:, t], start=(t == 0), stop=(t == 5))
                    ot = work.tile([128, CH], F32, tag="ot")
                    nc.vector.tensor_copy(ot[:], ps[:])
                    nc.sync.dma_start(outp[bass.ds(c * CH + m * 128, 128),
                                           bass.ts(n, CH)], ot[:])
    return nc


def _perm96():
    """per-head column order: rot1 | pass(32:48) | rot2 | pass(48:80)"""
    return np.concatenate([
        np.arange(0, 16),        # rot first halves  (d 0..15)
        np.arange(32, 48),       # pass
        np.arange(16, 32),       # rot second halves (d 16..31)
        np.arange(48, 80),       # pass
    ])


def _host_inputs(hidden_states, position_ids, Wqkv, bqkv, Wd):
    hs = np.ascontiguousarray(np.asarray(hidden_states, dtype=np.float32))
    pos = np.asarray(position_ids).reshape(-1).astype(np.float64)
    Wqkv = np.asarray(Wqkv, dtype=np.float32)
    bqkv = np.asarray(bqkv, dtype=np.float32)
    Wd = np.asarray(Wd, dtype=np.float32)

    inv = 1.0 / (10000.0 ** (np.arange(0, 32, 2, dtype=np.float64) / 32.0))
    fr = pos[:, None] * inv[None, :]          # [S, 16]
    cosT = np.ascontiguousarray(np.cos(fr).astype(np.float32).T)
    sinT = np.ascontiguousarray(np.sin(fr).astype(np.float32).T)

    maskb = np.ascontiguousarray(
        (np.arange(896)[None, :] >= (np.arange(128)[:, None] + 384))
        .astype(np.float32))

    xTs = [np.ascontiguousarray(hs[b].T) for b in range(hs.shape[0])]
    perm = _perm96()
    vpad_np = np.zeros((128, 8 * 17), dtype=np.float32)
    vpad_np[:, 16::17] = 1.0
    vpad_np = np.ascontiguousarray(vpad_np)

    in_maps = []
    for core in range(8):
        b, g = core // 4, core % 4
        heads = np.arange(8 * g, 8 * g + 8)
        wqk_c = np.zeros((HIDDEN, 1536), dtype=np.float32)
        bqk_c = np.zeros(1536, dtype=np.float32)
        for j, h in enumerate(heads):
            qsrc = h * 240 + perm
            ksrc = h * 240 + 80 + perm
            wqk_c[:, j * HP:j * HP + 80] = Wqkv[:, qsrc]
            wqk_c[:, 768 + j * HP:768 + j * HP + 80] = Wqkv[:, ksrc]
            bqk_c[j * HP:j * HP + 80] = bqkv[qsrc]
            bqk_c[768 + j * HP:768 + j * HP + 80] = bqkv[ksrc]
        vcols = (heads[:, None] * 240 + 160 + np.arange(80)[None, :]).reshape(-1)
        wd_c = np.zeros((768, HIDDEN), dtype=np.float32)
        for j, h in enumerate(heads):
            wd_c[j * HP:j * HP + 80] = Wd[h * 80:h * 80 + 80, :]
        in_maps.append({
            "xT": xTs[b],
            "wqk": np.ascontiguousarray(wqk_c),
            "wv": np.ascontiguousarray(Wqkv[:, vcols]),
            "wd": wd_c,
            "bqk": np.ascontiguousarray(bqk_c.reshape(NQK, 128).T),
            "bv": np.ascontiguousarray(bqkv[vcols].reshape(1, 640)),
            "cosT": cosT, "sinT": sinT, "maskb": maskb, "vpad": vpad_np,
        })
    return in_maps


_CACHE = {}


def kernel(hidden_states, position_ids, Wqkv, bqkv, Wd, bd):
    in_maps = _host_inputs(hidden_states, position_ids, Wqkv, bqkv, Wd)
    if "nc" not in _CACHE:
        nc = _build()
        nc.finalize()
        _CACHE["nc"] = nc
    res = run_bass_kernel_spmd(_CACHE["nc"], in_maps, core_ids=list(range(8)))
    B = np.asarray(hidden_states).shape[0]
    out = np.zeros((B, S, HIDDEN), dtype=np.float32)
    for core in range(8):
        out[core // 4] += res.results[core]["outp"]
    out += np.asarray(bd, dtype=np.float32)[None, None, :]
    return out
